# revision 3
# baseline (speedup 1.0000x reference)
"""BiDAF attention (nn_BertBidafAttention) on 8 TRN2 NeuronCores.

Math (per batch, reference):
    cp = c @ W.T + b            [CL, H]
    s  = cp @ q.T               [CL, QL]
    s1 = softmax_q(s + qmask_bias)      (row softmax)
    s2 = softmax_c(s + cmask_bias)      (col softmax)
    a  = s1 @ q                 [CL, H]
    bv = (s1 @ s2.T) @ c        [CL, H]
    x  = [c, a, c*a, c*bv]      [CL, 4H]

Restructured for the PE:
    qW[q,h] = sum_d q[q,d] W[d,h]       (75 MF instead of 604 MF for cp)
    sT[q,c] = sum_h qW[q,h] cT[h,c] + cmask_bias[c]   (one layout only)
    s[c,q]  = PE-transpose(sT) + (qb + qmask_bias)[q]  (rank-1 accumulate)
    bv      = s1 @ (s2.T @ c)           (avoids the [CL,CL] product)
The c-mask bias rides along into s but is constant along q for fixed c, so
it cancels in s1's softmax; likewise qb cancels inside the column softmax
s2.  NEGB = -1000 guarantees exp(masked - max) == 0 exactly in fp32 while
keeping the bias-shifted logits' fp32 quantization error ~7e-5.

float32r: the PE runs fp32 data at full rate (vs 4 cycles/row) when the
free dim is >= 256, with TF32-like operand rounding.  The BIR verifier
requires f32r matmul operands to be *produced* as f32r, so tiles feeding
f32r matmuls are declared f32r (DMA/copy producers then satisfy the
check); fp32 consumers of the same data read .bitcast(f32) views.
precision tiers: 0 = all fp32, 1 = f32r for post-softmax value matmuls
(a, bv, qc), 2 = also f32r for the logit matmul sT.

Sharding: data-parallel over batch, 2 batches per core, no collectives.
"""

import numpy as np
from contextlib import ExitStack, nullcontext as _nullctx

import concourse.bass as bass
from concourse import bacc
import concourse.mybir as mybir
import concourse.tile as tile
from concourse.masks import make_identity
from concourse.bass_utils import run_bass_kernel_spmd

B, CL, QL, H = 16, 512, 64, 768
NCORES = 8
BPC = B // NCORES  # batches per core
HK = H // 128      # 6 k-tiles over the feature dims
CT = CL // 128     # 4 c-tiles
NH = H // 2        # 384, N per matmul half (<=512 fp32 moving limit)
NEGB = -1000.0     # additive mask bias; exp(NEGB - max) == 0.0 in fp32

f32 = mybir.dt.float32
f32r = mybir.dt.float32r
i32 = mybir.dt.int32


def _build_nc(precision: int = 1, repeat: int = 1, hwloop: int = 0) -> bass.Bass:
    nc = bacc.Bacc()
    cD = nc.declare_dram_parameter("c", [BPC, CL, H], f32, isOutput=False)
    qD = nc.declare_dram_parameter("q", [BPC, QL, H], f32, isOutput=False)
    cmD = nc.declare_dram_parameter("c_mask", [BPC, CL], i32, isOutput=False)
    qmD = nc.declare_dram_parameter("q_mask", [BPC, QL], i32, isOutput=False)
    WD = nc.declare_dram_parameter("W", [H, H], f32, isOutput=False)
    bD = nc.declare_dram_parameter("b", [H], f32, isOutput=False)
    outD = nc.declare_dram_parameter("out", [BPC, CL, 4 * H], f32, isOutput=True)

    vdt = f32r if precision >= 1 else f32   # value-path matmul dtype
    ldt = f32r if precision >= 2 else f32   # logit-path matmul dtype
    ddt = f32r if precision >= 2 else f32   # bulk data tiles (c, q, s2)

    def asf32(ap):
        return ap.bitcast(f32) if ap.dtype != f32 else ap

    with tile.TileContext(nc) as tc, ExitStack() as ctx:
        const = ctx.enter_context(tc.tile_pool(name="const", bufs=1))
        wpool = ctx.enter_context(tc.tile_pool(name="wpool", bufs=1))
        perb = ctx.enter_context(tc.tile_pool(name="perb", bufs=2))
        small = ctx.enter_context(tc.tile_pool(name="small", bufs=2))
        outp = ctx.enter_context(tc.tile_pool(name="outp", bufs=3))
        ptp = ctx.enter_context(tc.tile_pool(name="ptp", bufs=2, space="PSUM"))
        pacc = ctx.enter_context(tc.tile_pool(name="pacc", bufs=4, space="PSUM"))

        ident = const.tile([128, 128], f32)
        make_identity(nc, ident)
        ones = const.tile([1, 128], f32)
        nc.vector.memset(ones, 1.0)

        # --- shared weights ---
        w_sb = wpool.tile([128, HK, H], f32)
        for k in range(HK):
            nc.scalar.dma_start(out=w_sb[:, k, :],
                                in_=WD[k * 128:(k + 1) * 128, :])
        b_sb = wpool.tile([128, HK], f32)
        nc.scalar.dma_start(out=b_sb, in_=bD[:].rearrange("(k p) -> p k", p=128))

        # --- mask bias rows (int32 -> fp32 cast during SWDGE DMA) ---
        qmfc = small.tile([QL, BPC], f32, tag="qmfc", bufs=1)
        nc.gpsimd.dma_start(out=qmfc, in_=qmD[:].rearrange("b l -> l b"))
        cmf = small.tile([1, BPC, CL], f32, tag="cmf", bufs=1)
        nc.gpsimd.dma_start(out=cmf[:1].rearrange("o b l -> o (b l)"),
                            in_=cmD[:].rearrange("b (o l) -> o (b l)", o=1))
        # bias = (mask - 1) * |NEGB|  ->  0 where mask==1, NEGB where mask==0
        qbias_c = small.tile([QL, BPC], f32, tag="qbias_c", bufs=1)
        nc.scalar.activation(qbias_c, qmfc, mybir.ActivationFunctionType.Copy,
                             bias=NEGB, scale=-NEGB)
        cbias = small.tile([1, BPC, CL], f32, tag="cbias", bufs=1)
        nc.scalar.activation(cbias, cmf, mybir.ActivationFunctionType.Copy,
                             bias=NEGB, scale=-NEGB)

        for _rep in range(repeat):
          with (tc.For_i(0, hwloop, 1) if hwloop > 1 else _nullctx()):
                # --- load q and c up front; qT/cT via PE transposes ---
                q_nat, q_r, c_nats, cTs = [], [], [], []
                qT2 = wpool.tile([128, HK, BPC, QL], f32)  # [d, k, b, q]
                for bi in range(BPC):
                    qn = perb.tile([64, H], ddt, tag="q_nat")
                    nc.sync.dma_start(out=qn, in_=qD[bi].bitcast(ddt))
                    q_nat.append(qn)
                    if vdt != ddt:
                        qr = perb.tile([64, H], vdt, tag="q_r")
                        nc.sync.dma_start(out=qr, in_=qD[bi].bitcast(vdt))
                        q_r.append(qr)
                    else:
                        q_r.append(qn)
                for bi in range(BPC):
                    c_nat = perb.tile([128, CT, H], ddt, tag="c_nat")
                    for ci in range(CT):
                        nc.sync.dma_start(
                            out=c_nat[:, ci, :],
                            in_=cD[bi, ci * 128:(ci + 1) * 128, :].bitcast(ddt))
                    c_nats.append(c_nat)
                for bi in range(BPC):
                    for k in range(HK):
                        tp = ptp.tile([128, QL], f32, tag="tp")
                        nc.tensor.transpose(
                            tp, asf32(q_nat[bi][:, k * 128:(k + 1) * 128]),
                            ident[:64, :64])
                        nc.scalar.copy(out=qT2[:, k, bi, :], in_=tp)
                for bi in range(BPC):
                    cT = perb.tile([128, HK, CL], ldt, tag="cT")
                    for ci in range(CT):
                        for k in range(HK):
                            tp = ptp.tile([128, 128], f32, tag="tp")
                            nc.tensor.transpose(
                                tp, asf32(c_nats[bi][:, ci, k * 128:(k + 1) * 128]),
                                ident)
                            nc.vector.tensor_copy(
                                out=cT[:, k, ci * 128:(ci + 1) * 128], in_=tp)
                    cTs.append(cT)

                # --- qWT[h, (b q)] = sum_d W[d,h] qT[d, (b q)], k-outer so each
                # W chunk is consumed as it lands; 6 accumulators packed into two
                # PSUM banks ---
                qwt = wpool.tile([128, HK, BPC * QL], ldt)  # [h, hm, (b q)]
                # q-side bias as a per-partition column in the sT layout:
                # qb[q] = sum_d q[q,d] b[d], plus the q-mask bias
                qrow_c = []
                for bi in range(BPC):
                    ps_qbc = pacc.tile([64, 1], f32, tag="acc")
                    for k in range(HK):
                        nc.tensor.matmul(ps_qbc, qT2[:, k, bi, :],
                                         b_sb[:, k:k + 1],
                                         start=(k == 0), stop=(k == HK - 1))
                    qrc = small.tile([64, 1], f32, tag="qrc", name=f"qrc{bi}")
                    nc.vector.tensor_add(qrc, ps_qbc, qbias_c[:, bi:bi + 1])
                    qrow_c.append(qrc)
                for wave in range(2):
                    hms = range(wave * 3, wave * 3 + 3)
                    ps_w = {hm: pacc.tile([128, BPC * QL], f32, tag="acc",
                                          name=f"ps_w{hm}") for hm in hms}
                    for k in range(HK):
                        rhs = qT2[:, k].rearrange("p b q -> p (b q)")
                        for hm in hms:
                            nc.tensor.matmul(ps_w[hm],
                                             w_sb[:, k, hm * 128:(hm + 1) * 128],
                                             rhs, start=(k == 0),
                                             stop=(k == HK - 1))
                    for hm in hms:
                        nc.scalar.copy(out=qwt[:, hm, :], in_=ps_w[hm])
                for bi in range(BPC):
                    c_nat = c_nats[bi]
                    cT = cTs[bi]
                    # --- sT[q, c] logits + c-mask bias (rank-1 accumulate) ---
                    ps_st = pacc.tile([64, CL], f32, tag="acc")
                    for k in range(HK):
                        nc.tensor.matmul(ps_st, qwt[:, k, bi * QL:(bi + 1) * QL],
                                         cT[:, k], start=(k == 0), stop=False)
                    nc.tensor.matmul(ps_st, ones[:1, :QL], cbias[:1, bi],
                                     start=False, stop=True)
                    # biased logits to SBUF for the s-path transposes
                    sTb = small.tile([64, CL], f32, tag="sTb")
                    nc.vector.tensor_scalar_add(sTb, ps_st, qrow_c[bi])

                    # --- column softmax s2 (over c, the free axis here) ---
                    nmax2 = small.tile([64, 1], f32, tag="nmax2")
                    nc.vector.reduce_max(nmax2, ps_st, axis=mybir.AxisListType.X,
                                         negate=True)
                    e2 = small.tile([64, CL], f32, tag="e2")
                    sum2 = small.tile([64, 1], f32, tag="sum2")
                    nc.scalar.activation(e2, ps_st,
                                         mybir.ActivationFunctionType.Exp,
                                         bias=nmax2, scale=1.0, accum_out=sum2)
                    r2 = small.tile([64, 1], f32, tag="r2")
                    nc.vector.reciprocal(r2, sum2)
                    s2T = small.tile([64, CL], f32, tag="s2T")
                    nc.vector.tensor_scalar_mul(s2T, e2, r2)
                    s2 = small.tile([128, CT, QL], ddt, tag="s2")
                    for ci in range(CT):
                        tp = ptp.tile([128, QL], f32, tag="tp")
                        nc.tensor.transpose(tp, s2T[:, ci * 128:(ci + 1) * 128],
                                            ident[:64, :64])
                        nc.vector.tensor_copy(out=s2[:, ci, :], in_=tp)

                    # --- row softmax s1: s = sTb.T + qrow, then softmax over q ---
                    s1T = small.tile([64, CL], vdt, tag="s1T")
                    for ci in range(CT):
                        ps_s = pacc.tile([128, QL], f32, tag="acc")
                        nc.tensor.matmul(ps_s, sTb[:, ci * 128:(ci + 1) * 128],
                                         ident[:64, :64], is_transpose=True,
                                         start=True, stop=True)
                        nmax1 = small.tile([128, 1], f32, tag="nmax1")
                        nc.vector.reduce_max(nmax1, ps_s,
                                             axis=mybir.AxisListType.X,
                                             negate=True)
                        e1 = small.tile([128, QL], f32, tag="e1")
                        sum1 = small.tile([128, 1], f32, tag="sum1")
                        nc.scalar.activation(e1, ps_s,
                                             mybir.ActivationFunctionType.Exp,
                                             bias=nmax1, scale=1.0,
                                             accum_out=sum1)
                        r1 = small.tile([128, 1], f32, tag="r1")
                        nc.vector.reciprocal(r1, sum1)
                        s1 = small.tile([128, QL], f32, tag="s1")
                        nc.vector.tensor_scalar_mul(s1, e1, r1)
                        tp = ptp.tile([64, 128], f32, tag="tp")
                        nc.tensor.transpose(tp, s1, ident)
                        nc.scalar.copy(out=s1T[:, ci * 128:(ci + 1) * 128], in_=tp)

                    # --- qc[q, h] = s2.T @ c ---
                    qc = perb.tile([64, H], vdt, tag="qc")
                    for hf in range(2):
                        ps_qc = pacc.tile([64, NH], f32, tag="acc")
                        for ci in range(CT):
                            nc.tensor.matmul(ps_qc, s2[:, ci, :],
                                             c_nat[:, ci, hf * NH:(hf + 1) * NH],
                                             start=(ci == 0), stop=(ci == CT - 1))
                        nc.scalar.copy(out=qc[:, hf * NH:(hf + 1) * NH], in_=ps_qc)

                    # --- a = s1 @ q ; bv = s1 @ qc ; outputs ---
                    for ci in range(CT):
                        rows = slice(ci * 128, (ci + 1) * 128)
                        a_sb = outp.tile([128, H], f32, tag="a")
                        ca_sb = outp.tile([128, H], f32, tag="ca")
                        cbv_sb = outp.tile([128, H], f32, tag="cbv")
                        for hf in range(2):
                            cols = slice(hf * NH, (hf + 1) * NH)
                            ps_a = pacc.tile([128, NH], f32, tag="acc")
                            nc.tensor.matmul(ps_a, s1T[:, rows], q_r[bi][:, cols],
                                             start=True, stop=True)
                            nc.scalar.copy(out=a_sb[:, cols], in_=ps_a)
                            nc.vector.tensor_mul(ca_sb[:, cols],
                                                 asf32(c_nat[:, ci, cols]),
                                                 a_sb[:, cols])
                            ps_bv = pacc.tile([128, NH], f32, tag="acc")
                            nc.tensor.matmul(ps_bv, s1T[:, rows], qc[:, cols],
                                             start=True, stop=True)
                            nc.vector.tensor_mul(cbv_sb[:, cols],
                                                 asf32(c_nat[:, ci, cols]),
                                                 ps_bv)
                        nc.sync.dma_start(out=outD[bi, rows, 0:H],
                                          in_=asf32(c_nat[:, ci, :]))
                        nc.sync.dma_start(out=outD[bi, rows, H:2 * H], in_=a_sb)
                        nc.sync.dma_start(out=outD[bi, rows, 2 * H:3 * H],
                                          in_=ca_sb)
                        nc.sync.dma_start(out=outD[bi, rows, 3 * H:4 * H],
                                          in_=cbv_sb)

    nc.finalize()
    return nc


_NC_CACHE: dict = {}


def _get_nc(precision: int = 1) -> bass.Bass:
    if precision not in _NC_CACHE:
        _NC_CACHE[precision] = _build_nc(precision)
    return _NC_CACHE[precision]


def kernel(c, q, c_mask, q_mask, W, b, _trace=False, _precision=1, _tmpdir=None):
    nc = _get_nc(_precision)
    in_maps = []
    for i in range(NCORES):
        sl = slice(i * BPC, (i + 1) * BPC)
        in_maps.append({
            "c": np.ascontiguousarray(np.asarray(c)[sl], dtype=np.float32),
            "q": np.ascontiguousarray(np.asarray(q)[sl], dtype=np.float32),
            "c_mask": np.ascontiguousarray(np.asarray(c_mask)[sl], dtype=np.int32),
            "q_mask": np.ascontiguousarray(np.asarray(q_mask)[sl], dtype=np.int32),
            "W": np.ascontiguousarray(np.asarray(W), dtype=np.float32),
            "b": np.ascontiguousarray(np.asarray(b), dtype=np.float32),
        })
    res = run_bass_kernel_spmd(nc, in_maps, core_ids=list(range(NCORES)),
                               trace=_trace, tmpdir=_tmpdir)
    out = np.concatenate([res.results[i]["out"] for i in range(NCORES)], axis=0)
    if _trace:
        return out, res
    return out



# revision 12
# speedup vs baseline: 1.2549x; 1.2549x over previous
"""BiDAF attention (nn_BertBidafAttention) on 8 TRN2 NeuronCores.

Math (per batch, reference):
    cp = c @ W.T + b            [CL, H]
    s  = cp @ q.T               [CL, QL]
    s1 = softmax_q(s + qmask_bias)      (row softmax)
    s2 = softmax_c(s + cmask_bias)      (col softmax)
    a  = s1 @ q                 [CL, H]
    bv = (s1 @ s2.T) @ c        [CL, H]
    x  = [c, a, c*a, c*bv]      [CL, 4H]

Restructured for the PE:
    qW[(b q), h'] = sum_h q[(b q), h] W[h, h']    (both batches fused, 75 MF)
    sT[q, c]      = sum_h qWT[h, q] cT[h, c] + cmask_bias[c]  (f32r, 512-wide)
    bv            = s1 @ (s2.T @ c)               (avoids the [CL,CL] product)
The c-mask bias is constant along q for fixed c so it cancels in s1's
softmax; the q-side bias (q.b + qmask_bias) is added per-partition into
sTb and cancels in s2's softmax.  NEGB=-1000 makes exp(masked-max)==0.

Precision split (validated vs the fp32 reference in numpy):
    logit matmuls (qW, sT) run f32r (TF32-ish rounding)  -> ~5e-3 rel
    value matmuls (a, bv, qc) + softmax weights run bf16 -> ~7e-3 rel total
    (bf16 logits would be 7e-2 and fail the 2e-2 gate; f32r values would
     be slower on DVE copies for no accuracy need.)
Softmaxes are kept unnormalized (e1, e2); the 1/sum factors are folded in
as per-partition scales later (e1*r1 before the s1 transpose, r2 on the
qc PSUM->SBUF copy), so no extra normalization passes exist.

Engine split: PE does transposes + matmuls; ScalarE does exp/bias-add and
PSUM->SBUF copies; DVE does reductions, reciprocals and c*bv; GpSimd does
the fp32->bf16 casts of c and c*a.  All outputs are fp32 in SBUF and
written with HWDGE DMAs (inputs on the sync ring, outputs on the scalar
ring so they don't head-of-line-block each other).

Sharding: data-parallel over batch, 2 batches per core, no collectives.
"""

import numpy as np
from contextlib import ExitStack

import concourse.bass as bass
from concourse import bacc
import concourse.mybir as mybir
import concourse.tile as tile
from concourse.masks import make_identity
from concourse.bass_utils import run_bass_kernel_spmd

B, CL, QL, H = 16, 512, 64, 768
NCORES = 8
BPC = B // NCORES  # batches per core
HK = H // 128      # 6 k-tiles over the feature dims
CT = CL // 128     # 4 c-tiles
NH = H // 2        # 384, N per matmul half (one PSUM bank)
NEGB = -1000.0     # additive mask bias; exp(NEGB - max) == 0.0 in fp32

f32 = mybir.dt.float32
f32r = mybir.dt.float32r
bf16 = mybir.dt.bfloat16
i32 = mybir.dt.int32


def _build_nc(precision: int = 1) -> bass.Bass:
    nc = bacc.Bacc()
    cD = nc.declare_dram_parameter("c", [BPC, CL, H], f32, isOutput=False)
    qD = nc.declare_dram_parameter("q", [BPC, QL, H], f32, isOutput=False)
    cmD = nc.declare_dram_parameter("c_mask", [BPC, CL], i32, isOutput=False)
    qmD = nc.declare_dram_parameter("q_mask", [BPC, QL], i32, isOutput=False)
    WD = nc.declare_dram_parameter("W", [H, H], f32, isOutput=False)
    bD = nc.declare_dram_parameter("b", [H], f32, isOutput=False)
    outD = nc.declare_dram_parameter("out", [BPC, CL, 4 * H], f32, isOutput=True)

    ldt = f32r if precision >= 1 else f32  # logit-path matmul dtype

    with tile.TileContext(nc) as tc, ExitStack() as ctx:
        const = ctx.enter_context(tc.tile_pool(name="const", bufs=1))
        wpool = ctx.enter_context(tc.tile_pool(name="wpool", bufs=1))
        cpool = ctx.enter_context(tc.tile_pool(name="cpool", bufs=1))
        small = ctx.enter_context(tc.tile_pool(name="small", bufs=2))
        outp = ctx.enter_context(tc.tile_pool(name="outp", bufs=3))
        ptp = ctx.enter_context(tc.tile_pool(name="ptp", bufs=3, space="PSUM"))
        pacc = ctx.enter_context(tc.tile_pool(name="pacc", bufs=2, space="PSUM"))
        pval = ctx.enter_context(tc.tile_pool(name="pval", bufs=3, space="PSUM"))

        ident = const.tile([128, 128], f32)
        make_identity(nc, ident)
        identb = const.tile([128, 128], bf16)
        nc.vector.tensor_copy(identb, ident)
        ones_f = const.tile([1, 128], f32)
        nc.vector.memset(ones_f, 1.0)
        if ldt == f32:
            ones = ones_f
        else:
            ones = const.tile([1, 128], ldt)
            nc.vector.tensor_copy(ones, ones_f)

        # ---- input DMAs (sync ring; W interleaved between c chunks so the
        # first cT transposes and the k-ordered qW accumulation start early)
        q_both = wpool.tile([128, H], ldt)  # [(b q), h]
        nc.sync.dma_start(out=q_both,
                          in_=qD[:].rearrange("b q h -> (b q) h").bitcast(ldt))
        c_f32s, w_order = [], []
        w_sb = wpool.tile([128, HK, H], ldt)  # [p, k, h']  (W[k*128+p, h'])
        for bi in range(BPC):
            c_f32s.append(cpool.tile([128, CT, H], f32, name=f"c{bi}"))
        dma_seq = []
        for ci in range(CT):
            dma_seq.append(("c", 0, ci))
            if ci < HK:
                dma_seq.append(("w", ci))
        for ci in range(CT):
            dma_seq.append(("c", 1, ci))
            if CT + ci < HK:
                dma_seq.append(("w", CT + ci))
        for item in dma_seq:
            if item[0] == "c":
                _, bi, ci = item
                nc.sync.dma_start(out=c_f32s[bi][:, ci, :],
                                  in_=cD[bi, ci * 128:(ci + 1) * 128, :])
            else:
                _, k = item
                nc.sync.dma_start(out=w_sb[:, k, :],
                                  in_=WD[k * 128:(k + 1) * 128, :].bitcast(ldt))
        b_sb = wpool.tile([128, HK], ldt)
        nc.sync.dma_start(out=b_sb,
                          in_=bD[:].rearrange("(k p) -> p k", p=128).bitcast(ldt))

        # bf16 copies of q (SWDGE cast straight from DRAM; per batch so the
        # partition base stays 0)
        q_bf = wpool.tile([64, BPC, H], bf16)
        for bi in range(BPC):
            nc.gpsimd.dma_start(out=q_bf[:, bi, :], in_=qD[bi])

        # ---- mask bias rows (int32 -> fp32 cast during SWDGE DMA) ----
        qmf = small.tile([64, BPC], f32, tag="qmf", bufs=1)
        nc.gpsimd.dma_start(out=qmf, in_=qmD[:].rearrange("b q -> q b"))
        cmf = small.tile([1, BPC, CL], f32, tag="cmf", bufs=1)
        nc.gpsimd.dma_start(out=cmf[:1].rearrange("o b l -> o (b l)"),
                            in_=cmD[:].rearrange("b (o l) -> o (b l)", o=1))
        # bias = (mask - 1) * |NEGB|  ->  0 where mask==1, NEGB where mask==0
        qmbias = small.tile([64, BPC], f32, tag="qmbias", bufs=1)
        nc.scalar.activation(qmbias, qmf, mybir.ActivationFunctionType.Copy,
                             bias=NEGB, scale=-NEGB)
        cbias = small.tile([1, BPC, CL], ldt, tag="cbias", bufs=1)
        nc.scalar.activation(cbias, cmf, mybir.ActivationFunctionType.Copy,
                             bias=NEGB, scale=-NEGB)

        # ---- qT[h, (b q)] via PE transposes of q_both ----
        qT = wpool.tile([128, HK, 128], ldt)
        for g in range(2):  # two groups of 3 k-chunks -> one PSUM bank each
            tp = ptp.tile([128, 3, 128], f32, tag="tp")
            for j in range(3):
                k = g * 3 + j
                nc.tensor.transpose(tp[:, j, :],
                                    q_both.bitcast(f32)[:, k * 128:(k + 1) * 128],
                                    ident)
            nc.vector.tensor_copy(out=qT[:, g * 3:(g + 1) * 3, :], in_=tp)

        # ---- qW[(b q), h'] = q @ W and qb[(b q)] = q . b ----
        qW = wpool.tile([128, H], ldt)
        for hf in range(2):
            ps_qw = pacc.tile([128, 512], f32, tag="acc")
            for k in range(HK):
                nc.tensor.matmul(ps_qw[:, :NH], qT[:, k, :],
                                 w_sb[:, k, hf * NH:(hf + 1) * NH],
                                 start=(k == 0), stop=(k == HK - 1))
            nc.scalar.copy(out=qW[:, hf * NH:(hf + 1) * NH], in_=ps_qw[:, :NH])
        # q-side bias as per-partition column per batch: [64, BPC]
        ps_qb = pacc.tile([64, BPC], f32, tag="acc")
        for bi in range(BPC):
            for k in range(HK):
                # N=1 violates the fp32r moving-dim ISA restriction; these
                # 12 tiny matmuls run as plain fp32 via bitcast views.
                nc.tensor.matmul(ps_qb[:, bi:bi + 1],
                                 qT[:, k, bi * 64:(bi + 1) * 64].bitcast(f32),
                                 b_sb[:, k:k + 1].bitcast(f32),
                                 start=(k == 0), stop=(k == HK - 1))
        qrow = small.tile([64, BPC], f32, tag="qrow", bufs=1)
        nc.vector.tensor_add(qrow, ps_qb, qmbias)

        # ---- qWT[h', (b q)] via PE transposes of qW ----
        qWT = wpool.tile([128, HK, 128], ldt)
        for g in range(2):
            tp = ptp.tile([128, 3, 128], f32, tag="tp")
            for j in range(3):
                k = g * 3 + j
                nc.tensor.transpose(tp[:, j, :],
                                    qW.bitcast(f32)[:, k * 128:(k + 1) * 128],
                                    ident)
            nc.scalar.copy(out=qWT[:, g * 3:(g + 1) * 3, :], in_=tp)

        for bi in range(BPC):
            c_f32 = c_f32s[bi]
            # out[:, 0:H] = c — no compute dependency, write immediately
            for ci in range(CT):
                rows = slice(ci * 128, (ci + 1) * 128)
                nc.scalar.dma_start(out=outD[bi, rows, 0:H],
                                    in_=c_f32[:, ci, :])

            # bf16 copy of c for the value-path matmuls / products (GpSimd)
            c_bf = cpool.tile([128, CT, H], bf16, name=f"cb{bi}", tag="cbf")
            for ci in range(CT):
                nc.gpsimd.tensor_copy(out=c_bf[:, ci, :], in_=c_f32[:, ci, :])

            # ---- cT[h, c] via PE transposes, grouped per c-chunk ----
            cT = cpool.tile([128, HK, CL], ldt, name=f"ct{bi}", tag="ctp")
            for ci in range(CT):
                for g in range(2):
                    tp = ptp.tile([128, 3, 128], f32, tag="tp")
                    for j in range(3):
                        k = g * 3 + j
                        nc.tensor.transpose(
                            tp[:, j, :],
                            c_f32[:, ci, k * 128:(k + 1) * 128], ident)
                    eng = nc.vector if (ci + g) % 2 == 0 else nc.scalar
                    if eng is nc.vector:
                        nc.vector.tensor_copy(
                            out=cT[:, g * 3:(g + 1) * 3,
                                   ci * 128:(ci + 1) * 128], in_=tp)
                    else:
                        nc.scalar.copy(
                            out=cT[:, g * 3:(g + 1) * 3,
                                   ci * 128:(ci + 1) * 128], in_=tp)

            # ---- logits sT[q, c] (f32r, 512-wide) + c-mask bias ----
            ps_st = pacc.tile([64, CL], f32, tag="acc")
            for k in range(HK):
                nc.tensor.matmul(ps_st, qWT[:, k, bi * 64:(bi + 1) * 64],
                                 cT[:, k, :], start=(k == 0), stop=False)
            nc.tensor.matmul(ps_st, ones[:1, :QL], cbias[:1, bi],
                             start=False, stop=True)

            # ---- column softmax s2 (over c = free axis), unnormalized ----
            nmax2 = small.tile([64, 1], f32, tag="nmax2")
            nc.vector.reduce_max(nmax2, ps_st, axis=mybir.AxisListType.X,
                                 negate=True)
            e2T = small.tile([64, CL], bf16, tag="e2T")
            sum2 = small.tile([64, 1], f32, tag="sum2")
            nc.scalar.activation(e2T, ps_st, mybir.ActivationFunctionType.Exp,
                                 bias=nmax2, scale=1.0, accum_out=sum2)
            r2 = small.tile([64, 1], f32, tag="r2")
            nc.vector.reciprocal(r2, sum2)
            # e2[c, q] chunks for the qc matmul (transpose back)
            tpe = ptp.tile([128, CT, QL], bf16, tag="tp")
            for ci in range(CT):
                nc.tensor.transpose(tpe[:, ci, :],
                                    e2T[:, ci * 128:(ci + 1) * 128],
                                    identb[:64, :64])
            e2s = small.tile([128, CT, QL], bf16, tag="e2s")
            nc.vector.tensor_copy(out=e2s, in_=tpe)

            # biased logits (+ q-side bias, per partition) for the s1 path
            sTb = small.tile([64, CL], f32, tag="sTb")
            nc.scalar.add(sTb, ps_st, qrow[:, bi:bi + 1])

            # ---- row softmax s1: transpose to [c, q], softmax over free q,
            # scale by 1/sum, transpose back to s1T[q, c] (bf16) ----
            tps = ptp.tile([128, CT, QL], f32, tag="tp")
            for ci in range(CT):
                nc.tensor.transpose(tps[:, ci, :],
                                    sTb[:, ci * 128:(ci + 1) * 128],
                                    ident[:64, :64])
            e1 = small.tile([128, CT, QL], bf16, tag="e1")
            sum1 = small.tile([128, CT], f32, tag="sum1")
            for ci in range(CT):
                nmax1 = small.tile([128, 1], f32, tag="nmax1")
                nc.vector.reduce_max(nmax1, tps[:, ci, :],
                                     axis=mybir.AxisListType.X, negate=True)
                nc.scalar.activation(e1[:, ci, :], tps[:, ci, :],
                                     mybir.ActivationFunctionType.Exp,
                                     bias=nmax1, scale=1.0,
                                     accum_out=sum1[:, ci:ci + 1])
            r1 = small.tile([128, CT], f32, tag="r1")
            nc.vector.reciprocal(r1, sum1)
            s1 = small.tile([128, CT, QL], bf16, tag="s1")
            for ci in range(CT):
                nc.vector.tensor_scalar_mul(s1[:, ci, :], e1[:, ci, :],
                                            r1[:, ci:ci + 1])
            tpt = ptp.tile([64, CL], bf16, tag="tp")
            for ci in range(CT):
                nc.tensor.transpose(tpt[:, ci * 128:(ci + 1) * 128],
                                    s1[:, ci, :], identb)
            s1T = small.tile([64, CL], bf16, tag="s1T")
            nc.vector.tensor_copy(out=s1T, in_=tpt)

            # ---- qc[q, h] = s2.T @ c  (bf16, scale r2 on copy-out) ----
            qc_bf = small.tile([64, H], bf16, tag="qc")
            for hf in range(2):
                ps_qc = pacc.tile([64, 512], f32, tag="acc")
                for ci in range(CT):
                    nc.tensor.matmul(ps_qc[:, :NH], e2s[:, ci, :],
                                     c_bf[:, ci, hf * NH:(hf + 1) * NH],
                                     start=(ci == 0), stop=(ci == CT - 1))
                nc.scalar.mul(qc_bf[:, hf * NH:(hf + 1) * NH],
                              ps_qc[:, :NH], r2)

            # ---- a = s1 @ q ; bv = s1 @ qc ; products; outputs ----
            for ci in range(CT):
                rows = slice(ci * 128, (ci + 1) * 128)
                a_f32 = outp.tile([128, H], f32, tag="af")
                cbv_f32 = outp.tile([128, H], f32, tag="cbvf")
                for hf in range(2):
                    cols = slice(hf * NH, (hf + 1) * NH)
                    ps_a = pval.tile([128, 512], f32, tag="val")
                    nc.tensor.matmul(ps_a[:, :NH],
                                     s1T[:, ci * 128:(ci + 1) * 128],
                                     q_bf[:, bi, cols], start=True, stop=True)
                    nc.scalar.copy(out=a_f32[:, cols], in_=ps_a[:, :NH])
                    ps_bv = pval.tile([128, 512], f32, tag="val")
                    nc.tensor.matmul(ps_bv[:, :NH],
                                     s1T[:, ci * 128:(ci + 1) * 128],
                                     qc_bf[:, cols], start=True, stop=True)
                    nc.vector.tensor_mul(cbv_f32[:, cols],
                                         c_f32[:, ci, cols], ps_bv[:, :NH])
                ca_f32 = outp.tile([128, H], f32, tag="caf")
                nc.gpsimd.tensor_mul(ca_f32, c_f32[:, ci, :], a_f32)
                nc.scalar.dma_start(out=outD[bi, rows, H:2 * H], in_=a_f32)
                nc.scalar.dma_start(out=outD[bi, rows, 2 * H:3 * H],
                                    in_=ca_f32)
                nc.scalar.dma_start(out=outD[bi, rows, 3 * H:4 * H],
                                    in_=cbv_f32)

    nc.finalize()
    return nc


_NC_CACHE: dict = {}


def _get_nc(precision: int = 1) -> bass.Bass:
    if precision not in _NC_CACHE:
        _NC_CACHE[precision] = _build_nc(precision)
    return _NC_CACHE[precision]


def kernel(c, q, c_mask, q_mask, W, b, _trace=False, _precision=1, _tmpdir=None):
    nc = _get_nc(_precision)
    in_maps = []
    for i in range(NCORES):
        sl = slice(i * BPC, (i + 1) * BPC)
        in_maps.append({
            "c": np.ascontiguousarray(np.asarray(c)[sl], dtype=np.float32),
            "q": np.ascontiguousarray(np.asarray(q)[sl], dtype=np.float32),
            "c_mask": np.ascontiguousarray(np.asarray(c_mask)[sl], dtype=np.int32),
            "q_mask": np.ascontiguousarray(np.asarray(q_mask)[sl], dtype=np.int32),
            "W": np.ascontiguousarray(np.asarray(W), dtype=np.float32),
            "b": np.ascontiguousarray(np.asarray(b), dtype=np.float32),
        })
    res = run_bass_kernel_spmd(nc, in_maps, core_ids=list(range(NCORES)),
                               trace=_trace, tmpdir=_tmpdir)
    out = np.concatenate([res.results[i]["out"] for i in range(NCORES)], axis=0)
    if _trace:
        return out, res
    return out
